# revision 8
# baseline (speedup 1.0000x reference)
"""Trainium2 Bass kernel for the ESM contrastive projection head loss.

Problem (hardcoded): x [512, 512, 960] f32; two 2-layer MLPs (codon for batch
rows 0:256, amino for 256:512) applied to mean-pooled x; pairwise cosine
similarity of the concatenated projections z [512, 240]; diag-masked,
temperature-scaled InfoNCE-style NLL, mean over rows.

Strategy: data-parallel over batch across 8 NeuronCores (64 rows each).
x is cast to fp16 on the host (mean-pooling over 512 makes the quantization
error negligible: measured rel err ~3e-6), halving the HBM stream to 63 MB
per core. Each core streams its shard, reduces the per-partition rows with
DVE tree-adds, and accumulates pooled^T directly in PSUM via per-chunk
matmuls against a sliding one-hot window that also applies the 1/512 mean.
The MLP runs in fp16 (biases folded in as K=1 ones-row matmuls). Row norms
are folded into z before the allgather (unit vectors are gathered), so the
similarity matmul yields logits directly; the diagonal is handled by the
identity cos_ii == 1: row max is exactly 1/T, exp uses a constant -10 bias,
and each row's own diag term is subtracted from the exp-sum. Activation
tables (Sqrt/Exp/Ln) are preloaded during streaming; constants load via the
gpsimd SWDGE queue so the x stream owns all HWDGE semaphore lanes.
Each core outputs nll [64,1]; the host averages.
"""
import contextlib
import ctypes
import os
import sys
import types

import numpy as np

B = 512
S = 512
D = 960
NCORES = 8
BPC = B // NCORES           # 64 batch rows per core
SLAB_B = 2                  # batch rows per DMA slab
NSLAB = BPC // SLAB_B       # 32
INV_T = 10.0                # 1 / temperature
D1 = D // 2                 # 480
D2 = D // 4                 # 240
NCHUNK = 8                  # 960 = 8 * 120 contraction chunks
CH = 120

_CACHE = {}
LAST_RESULT = None
TRACE_CORES = [0]


def _install_ntff_hook():
    """Make run_bass_kernel_spmd(trace=True) work under axon (test.py only)."""
    if "antenv.axon_hooks" in sys.modules:
        return
    so_path = "/opt/axon/libaxon_pjrt.so"
    try:
        lib = ctypes.CDLL(so_path)
    except OSError:
        return
    if not hasattr(lib, "axon_start_nrt_profile"):
        return
    lib.axon_start_nrt_profile.argtypes = [ctypes.POINTER(ctypes.c_int64), ctypes.c_size_t]
    lib.axon_start_nrt_profile.restype = ctypes.c_int64
    lib.axon_stop_nrt_profile.argtypes = [ctypes.c_char_p]
    lib.axon_stop_nrt_profile.restype = ctypes.c_int64

    @contextlib.contextmanager
    def _hook(output_dir, device_ids):
        import jax
        jax.devices()
        if device_ids:
            ids = (ctypes.c_int64 * len(device_ids))(*device_ids)
            rc = lib.axon_start_nrt_profile(ids, len(device_ids))
        else:
            rc = lib.axon_start_nrt_profile(None, 0)
        if rc != 0:
            raise RuntimeError(f"axon_start_nrt_profile rc={rc}")
        try:
            yield
        finally:
            n = lib.axon_stop_nrt_profile(str(output_dir).encode())
            print(f"profile: {n} file(s) written to {output_dir}", file=sys.stderr)

    mod = types.ModuleType("antenv.axon_hooks")
    mod.get_axon_ntff_profile_hook = lambda: _hook
    mod.set_axon_ntff_profile_hook = lambda h: None
    sys.modules["antenv.axon_hooks"] = mod


def _build_nc():
    import concourse.tile as tile
    from concourse import bacc, mybir

    f32 = mybir.dt.float32
    f16 = mybir.dt.float16
    add = mybir.AluOpType.add
    mult = mybir.AluOpType.mult
    sub = mybir.AluOpType.subtract
    AF = mybir.ActivationFunctionType

    nc = bacc.Bacc("TRN2", target_bir_lowering=False, debug=False,
                   enable_asserts=False, num_devices=NCORES)

    xs = nc.dram_tensor("xs", [BPC, S, D], f16, kind="ExternalInput").ap()
    w1 = nc.dram_tensor("w1", [D, D1], f16, kind="ExternalInput").ap()
    b1 = nc.dram_tensor("b1", [1, D1], f16, kind="ExternalInput").ap()
    w2 = nc.dram_tensor("w2", [D1, D2], f16, kind="ExternalInput").ap()
    b2 = nc.dram_tensor("b2", [1, D2], f16, kind="ExternalInput").ap()
    jwin = nc.dram_tensor("jwin", [128, 128], f16, kind="ExternalInput").ap()
    ident = nc.dram_tensor("ident", [128, 128], f16, kind="ExternalInput").ap()
    posm = nc.dram_tensor("posm", [BPC, B], f32, kind="ExternalInput").ap()
    out = nc.dram_tensor("nll", [BPC, 1], f32, kind="ExternalOutput").ap()

    with tile.TileContext(nc) as tc:
        with contextlib.ExitStack() as ctx:
            ep = ctx.enter_context
            consts = ep(tc.tile_pool(name="consts", bufs=1))
            xpool = ep(tc.tile_pool(name="xslab", bufs=10))
            spool = ep(tc.tile_pool(name="small", bufs=1))
            scr = ep(tc.tile_pool(name="scratch", bufs=1))
            dram = ep(tc.tile_pool(name="dram", bufs=1, space="DRAM"))
            ppT = ep(tc.tile_pool(name="ppT", bufs=1, space="PSUM"))
            psA = ep(tc.tile_pool(name="psA", bufs=1, space="PSUM"))
            pst = ep(tc.tile_pool(name="pst", bufs=2, space="PSUM"))

            # --- constant loads on the gpsimd SWDGE queue: keeps every HWDGE
            # semaphore lane free for the x stream (HWDGE lanes are shared
            # round-robin and the baseline's x slabs serialized behind the
            # const loads for the first ~40us) ---
            jwin_sb = consts.tile([128, 128], f16, tag="jwin")
            nc.gpsimd.dma_start(jwin_sb[:], jwin)
            ident_sb = consts.tile([128, 128], f16, tag="ident")
            nc.gpsimd.dma_start(ident_sb[:], ident)
            w1_sb = consts.tile([CH, NCHUNK, D1], f16, tag="w1")
            nc.gpsimd.dma_start(w1_sb[:], w1.rearrange("(k p) j -> p k j", p=CH))
            w2_sb = consts.tile([CH, 4, D2], f16, tag="w2")
            nc.gpsimd.dma_start(w2_sb[:], w2.rearrange("(k p) j -> p k j", p=CH))
            b1_sb = consts.tile([1, D1], f16, tag="b1")
            nc.gpsimd.dma_start(b1_sb[:], b1)
            b2_sb = consts.tile([1, D2], f16, tag="b2")
            nc.gpsimd.dma_start(b2_sb[:], b2)
            posm_sb = consts.tile([BPC, B], f32, tag="posm")
            nc.gpsimd.dma_start(posm_sb[:], posm)

            ones_sb = consts.tile([1, BPC], f16, tag="ones")
            nc.vector.memset(ones_sb[:], 1.0)
            mbias = consts.tile([BPC, 1], f32, tag="mbias")
            nc.vector.memset(mbias[:], -INV_T)

            # preload Sqrt/Exp/Ln activation tables while streaming runs
            tld = spool.tile([1, 1], f32, tag="tld")
            nc.vector.memset(tld[:], 1.0)
            tld2 = spool.tile([1, 1], f32, tag="tld2")
            nc.scalar.sqrt(tld2[:], tld[:])
            nc.scalar.activation(tld2[:], tld[:], AF.Exp)
            nc.scalar.activation(tld2[:], tld[:], AF.Ln)

            # --- warm up the collective path early (junk payload, result
            # folded into the output with x0 so it is not dead-code) ---
            wb = dram.tile([BPC, 8], f32, tag="wb")
            wg = dram.tile([B, 8], f32, tag="wg")
            nc.gpsimd.dma_start(wb[:], posm_sb[:, 0:8])
            nc.gpsimd.collective_compute(
                "AllGather", mybir.AluOpType.bypass,
                replica_groups=[list(range(NCORES))],
                ins=[wb.opt()], outs=[wg.opt()],
            )
            wg_sb = spool.tile([BPC, 1], f32, tag="wg")
            nc.gpsimd.dma_start(wg_sb[:], wg[0:BPC, 0:1])

            zb = dram.tile([D2, BPC], f16, tag="zb")
            zg = dram.tile([D2 * NCORES, BPC], f16, tag="zg")

            # --- phase A: stream x fp16, reduce to pooled^T in PSUM ---
            # slab: partition p holds 8 consecutive seq rows of batch row
            # p//64; DVE tree-adds reduce the 8 rows, then 8 chunk matmuls
            # against the sliding one-hot jwin (values 1/512) accumulate
            # pooled^T[d, b] across slabs. All 8 chunk regions share one
            # PSUM bank: start=True clears has_written for the whole bank,
            # so only the very first matmul starts; later chunks' first
            # writes overwrite-where-clear, everything after accumulates.
            pT_ps = ppT.tile([CH, NCHUNK, BPC], f32, tag="pT")
            for i in range(NSLAB):
                slab = xpool.tile([128, 8, D], f16, tag="slab")
                src = xs[SLAB_B * i:SLAB_B * (i + 1)].rearrange(
                    "b (q m) d -> (b q) m d", m=8)
                nc.sync.dma_start(slab[:], src)
                nc.vector.tensor_tensor(slab[:, 0:4, :], slab[:, 0:4, :],
                                        slab[:, 4:8, :], add)
                nc.vector.tensor_tensor(slab[:, 0:2, :], slab[:, 0:2, :],
                                        slab[:, 2:4, :], add)
                nc.vector.tensor_tensor(slab[:, 0, :], slab[:, 0, :],
                                        slab[:, 1, :], add)
                jw = jwin_sb[:, 64 - SLAB_B * i:128 - SLAB_B * i]
                for c in range(NCHUNK):
                    nc.tensor.matmul(pT_ps[:, c, :], slab[:, 0, CH * c:CH * (c + 1)],
                                     jw, start=(i == 0 and c == 0),
                                     stop=(i == NSLAB - 1),
                                     skip_group_check=True)

            pT_sb = spool.tile([CH, NCHUNK, BPC], f16, tag="pT")
            nc.vector.tensor_copy(pT_sb[:], pT_ps[:])

            # --- MLP layer 1: h [64, 480] = relu(pooled @ W1 + b1) ---
            h_ps = psA.tile([BPC, D1], f32, tag="h")
            for c in range(NCHUNK):
                nc.tensor.matmul(h_ps[:], pT_sb[:, c, :], w1_sb[:, c, :],
                                 start=(c == 0), stop=False)
            nc.tensor.matmul(h_ps[:], ones_sb[:], b1_sb[:],
                             start=False, stop=True)
            h_sb = spool.tile([BPC, D1], f16, tag="h")
            nc.vector.tensor_scalar_max(h_sb[:], h_ps[:], 0.0)

            # --- h^T via PE transposes ---
            hT_sb = spool.tile([CH, 4, BPC], f16, tag="hT")
            for t in range(4):
                tp = pst.tile([CH, BPC], f16, tag="tp")
                nc.tensor.transpose(tp[:], h_sb[:, CH * t:CH * (t + 1)],
                                    ident_sb[0:BPC, 0:BPC])
                nc.vector.tensor_copy(hT_sb[:, t, :], tp[:])

            # --- MLP layer 2: z [64, 240] = h @ W2 + b2 ---
            z_ps = psA.tile([BPC, D2], f32, tag="z")
            for t in range(4):
                nc.tensor.matmul(z_ps[:], hT_sb[:, t, :], w2_sb[:, t, :],
                                 start=(t == 0), stop=False)
            nc.tensor.matmul(z_ps[:], ones_sb[:], b2_sb[:],
                             start=False, stop=True)

            # --- fold 1/|z| into z: gather unit rows ---
            # (copy z to SBUF first: avoids a double-PSUM-read square and
            # the Square activation table; only Sqrt/Exp/Ln stay resident)
            z_sb = spool.tile([BPC, D2], f32, tag="zsb")
            nc.vector.tensor_copy(z_sb[:], z_ps[:])
            zsq = scr.tile([BPC, D2], f32, tag="zsq")
            nsq = spool.tile([BPC, 1], f32, tag="nsq")
            nc.vector.scalar_tensor_tensor(zsq[:], z_sb[:], 1.0, z_sb[:],
                                           mult, mult, accum_out=nsq[:])
            nrm = spool.tile([BPC, 1], f32, tag="nrm")
            nc.scalar.sqrt(nrm[:], nsq[:])
            ninv = spool.tile([BPC, 1], f32, tag="ninv")
            nc.vector.reciprocal(ninv[:], nrm[:])
            z1_sb = spool.tile([BPC, D2], f16, tag="z1")
            nc.vector.tensor_scalar_mul(z1_sb[:], z_sb[:], ninv[:])

            # --- z1^T [120, 2, 64]; allgather payload rows 0:240 = z1^T ---
            z1T_sb = spool.tile([CH, 2, BPC], f16, tag="z1T")
            for g in range(2):
                tp = pst.tile([CH, BPC], f16, tag="tp")
                nc.tensor.transpose(tp[:], z1_sb[:, CH * g:CH * (g + 1)],
                                    ident_sb[0:BPC, 0:BPC])
                nc.vector.tensor_copy(z1T_sb[:, g, :], tp[:])
            nc.sync.dma_start(
                zb[0:D2, :].rearrange("(og p) b -> p og b", p=CH), z1T_sb[:])
            nc.gpsimd.collective_compute(
                "AllGather", mybir.AluOpType.bypass,
                replica_groups=[list(range(NCORES))],
                ins=[zb.opt()], outs=[zg.opt()],
            )

            # own rows scaled by 1/T while the allgather runs; also the
            # exact diagonal term d2 = (10 z1) . z1 per own row
            z10T_sb = spool.tile([CH, 2, BPC], f16, tag="z10T")
            nc.vector.tensor_scalar_mul(z10T_sb[:], z1T_sb[:], INV_T)
            z10r = spool.tile([BPC, D2], f16, tag="z10r")
            nc.vector.tensor_scalar_mul(z10r[:], z1_sb[:], INV_T)
            dsq = scr.tile([BPC, D2], f32, tag="dsq")
            d2 = spool.tile([BPC, 1], f32, tag="d2")
            nc.vector.scalar_tensor_tensor(dsq[:], z10r[:], 1.0, z1_sb[:],
                                           mult, mult, accum_out=d2[:])
            dexp = spool.tile([BPC, 1], f32, tag="dexp")
            nc.scalar.activation(dexp[:], d2[:], AF.Exp, bias=mbias[:])

            # --- gathered z1^T -> [120, 2, 512] ---
            zgv = zg[:].rearrange("(c r) b -> r c b", r=D2)
            zfT_sb = spool.tile([CH, 2, B], f16, tag="zfT")
            for g in range(2):
                nc.sync.dma_start(
                    zfT_sb[:, g, :].rearrange("p (c b) -> p c b", b=BPC),
                    zgv[CH * g:CH * (g + 1)])

            # --- logits [64, 512] = (10 z1_own) @ z1_all^T, diag == 10 ---
            s_ps = psA.tile([BPC, B], f32, tag="sp")
            nc.tensor.matmul(s_ps[:], z10T_sb[:, 0, :], zfT_sb[:, 0, :],
                             start=True, stop=False)
            nc.tensor.matmul(s_ps[:], z10T_sb[:, 1, :], zfT_sb[:, 1, :],
                             start=False, stop=True)

            # --- nll = -pos + 10 + ln(sum_j exp(l_ij - 10) - diag) ---
            pos = spool.tile([BPC, 1], f32, tag="pos")
            e_sb = scr.tile([BPC, B], f32, tag="esb")
            nc.vector.scalar_tensor_tensor(e_sb[:], s_ps[:], 1.0, posm_sb[:],
                                           mult, mult, accum_out=pos[:])
            esum = spool.tile([BPC, 1], f32, tag="esum")
            e2_sb = scr.tile([BPC, B], f32, tag="e2sb")
            nc.scalar.activation(e2_sb[:], s_ps[:], AF.Exp,
                                 bias=mbias[:], scale=1.0, accum_out=esum[:])
            nc.vector.tensor_tensor(esum[:], esum[:], dexp[:], sub)
            lnv = spool.tile([BPC, 1], f32, tag="lnv")
            nc.scalar.activation(lnv[:], esum[:], AF.Ln)
            nll_sb = spool.tile([BPC, 1], f32, tag="nll")
            nc.vector.tensor_tensor(nll_sb[:], lnv[:], pos[:], sub)
            nc.vector.tensor_scalar_add(nll_sb[:], nll_sb[:], INV_T)
            # keep the warmup collective alive: nll += 0 * wg
            nc.vector.scalar_tensor_tensor(nll_sb[:], wg_sb[:], 0.0, nll_sb[:],
                                           mult, add)

            nc.sync.dma_start(out, nll_sb[:])

    nc.compile()
    return nc


def _host_inputs(x, W1c, b1c, W2c, b2c, W1a, b1a, W2a, b2a):
    x16 = np.asarray(x).astype(np.float16)
    # window matrix: slab i selects columns [64-2i, 128-2i); partition p
    # (batch half p//64) hits output column 2i + p//64. 1/512 applies the
    # mean (exact in fp16).
    jwin = np.zeros((128, 128), dtype=np.float16)
    jwin[np.arange(128), 64 + np.arange(128) // 64] = np.float16(1.0 / S)
    ident = np.eye(128, dtype=np.float16)
    in_maps = []
    for c in range(NCORES):
        rows = np.arange(BPC)
        gl = BPC * c + rows
        posm = np.zeros((BPC, B), dtype=np.float32)
        posm[rows, (gl + B // 2) % B] = 1.0
        if c < NCORES // 2:
            w1, bb1, w2, bb2 = W1c, b1c, W2c, b2c
        else:
            w1, bb1, w2, bb2 = W1a, b1a, W2a, b2a
        in_maps.append({
            "xs": x16[BPC * c:BPC * (c + 1)],
            "w1": np.ascontiguousarray(np.asarray(w1).astype(np.float16)),
            "b1": np.asarray(bb1).astype(np.float16).reshape(1, D1),
            "w2": np.ascontiguousarray(np.asarray(w2).astype(np.float16)),
            "b2": np.asarray(bb2).astype(np.float16).reshape(1, D2),
            "jwin": jwin,
            "ident": ident,
            "posm": posm,
        })
    return in_maps


def kernel(x, W1c, b1c, W2c, b2c, W1a, b1a, W2a, b2a):
    global LAST_RESULT
    trace = bool(os.environ.get("BASS_TRACE"))
    if trace:
        _install_ntff_hook()
    from concourse import bass_utils
    if trace:
        bass_utils.upload_artifacts = lambda tmpdir: "local://skipped"

    if "nc" not in _CACHE:
        _CACHE["nc"] = _build_nc()
    nc = _CACHE["nc"]

    in_maps = _host_inputs(x, W1c, b1c, W2c, b2c, W1a, b1a, W2a, b2a)
    kwargs = {}
    if trace:
        kwargs = {"trace": True, "trace_cores": TRACE_CORES}
    res = bass_utils.run_bass_kernel_spmd(
        nc, in_maps, list(range(NCORES)), **kwargs)
    LAST_RESULT = res
    nll = np.concatenate([res.results[c]["nll"][:, 0] for c in range(NCORES)])
    return np.asarray(nll.mean(dtype=np.float64), dtype=np.float32)


# revision 9
# speedup vs baseline: 1.0664x; 1.0664x over previous
"""Trainium2 Bass kernel for the ESM contrastive projection head loss.

Problem (hardcoded): x [512, 512, 960] f32; two 2-layer MLPs (codon for batch
rows 0:256, amino for 256:512) applied to mean-pooled x; pairwise cosine
similarity of the concatenated projections z [512, 240]; diag-masked,
temperature-scaled InfoNCE-style NLL, mean over rows.

Strategy: data-parallel over batch across 8 NeuronCores (64 rows each).
x is cast to fp16 on the host (mean-pooling over 512 makes the quantization
error negligible: measured rel err ~3e-6), halving the HBM stream to 63 MB
per core. Each core streams its shard, reduces the per-partition rows with
DVE tree-adds, and accumulates pooled^T directly in PSUM via per-chunk
matmuls against a sliding one-hot window that also applies the 1/512 mean.
The MLP runs in fp16 (biases folded in as K=1 ones-row matmuls). Row norms
are folded into z before the allgather (unit vectors are gathered), so the
similarity matmul yields logits directly; the diagonal is handled by the
identity cos_ii == 1: row max is exactly 1/T, exp uses a constant -10 bias,
and each row's own diag term is subtracted from the exp-sum. Activation
tables (Sqrt/Exp/Ln) are preloaded during streaming; constants load via the
gpsimd SWDGE queue so the x stream owns all HWDGE semaphore lanes.
Each core outputs nll [64,1]; the host averages.
"""
import contextlib
import ctypes
import os
import sys
import types

import numpy as np

B = 512
S = 512
D = 960
NCORES = 8
BPC = B // NCORES           # 64 batch rows per core
SLAB_B = 2                  # batch rows per DMA slab
NSLAB = BPC // SLAB_B       # 32
INV_T = 10.0                # 1 / temperature
D1 = D // 2                 # 480
D2 = D // 4                 # 240
NCHUNK = 8                  # 960 = 8 * 120 contraction chunks
CH = 120

_CACHE = {}
LAST_RESULT = None
TRACE_CORES = [0]


def _install_ntff_hook():
    """Make run_bass_kernel_spmd(trace=True) work under axon (test.py only)."""
    if "antenv.axon_hooks" in sys.modules:
        return
    so_path = "/opt/axon/libaxon_pjrt.so"
    try:
        lib = ctypes.CDLL(so_path)
    except OSError:
        return
    if not hasattr(lib, "axon_start_nrt_profile"):
        return
    lib.axon_start_nrt_profile.argtypes = [ctypes.POINTER(ctypes.c_int64), ctypes.c_size_t]
    lib.axon_start_nrt_profile.restype = ctypes.c_int64
    lib.axon_stop_nrt_profile.argtypes = [ctypes.c_char_p]
    lib.axon_stop_nrt_profile.restype = ctypes.c_int64

    @contextlib.contextmanager
    def _hook(output_dir, device_ids):
        import jax
        jax.devices()
        if device_ids:
            ids = (ctypes.c_int64 * len(device_ids))(*device_ids)
            rc = lib.axon_start_nrt_profile(ids, len(device_ids))
        else:
            rc = lib.axon_start_nrt_profile(None, 0)
        if rc != 0:
            raise RuntimeError(f"axon_start_nrt_profile rc={rc}")
        try:
            yield
        finally:
            n = lib.axon_stop_nrt_profile(str(output_dir).encode())
            print(f"profile: {n} file(s) written to {output_dir}", file=sys.stderr)

    mod = types.ModuleType("antenv.axon_hooks")
    mod.get_axon_ntff_profile_hook = lambda: _hook
    mod.set_axon_ntff_profile_hook = lambda h: None
    sys.modules["antenv.axon_hooks"] = mod


def _build_nc():
    import concourse.tile as tile
    from concourse import bacc, mybir

    f32 = mybir.dt.float32
    f16 = mybir.dt.float16
    add = mybir.AluOpType.add
    mult = mybir.AluOpType.mult
    sub = mybir.AluOpType.subtract
    AF = mybir.ActivationFunctionType

    nc = bacc.Bacc("TRN2", target_bir_lowering=False, debug=False,
                   enable_asserts=False, num_devices=NCORES)

    xs = nc.dram_tensor("xs", [BPC, S, D], f16, kind="ExternalInput").ap()
    w1 = nc.dram_tensor("w1", [D, D1], f16, kind="ExternalInput").ap()
    b1 = nc.dram_tensor("b1", [1, D1], f16, kind="ExternalInput").ap()
    w2 = nc.dram_tensor("w2", [D1, D2], f16, kind="ExternalInput").ap()
    b2 = nc.dram_tensor("b2", [1, D2], f16, kind="ExternalInput").ap()
    jwin = nc.dram_tensor("jwin", [128, 128], f16, kind="ExternalInput").ap()
    ident = nc.dram_tensor("ident", [128, 128], f16, kind="ExternalInput").ap()
    posm = nc.dram_tensor("posm", [BPC, B], f32, kind="ExternalInput").ap()
    out = nc.dram_tensor("nll", [BPC, 1], f32, kind="ExternalOutput").ap()

    with tile.TileContext(nc) as tc:
        with contextlib.ExitStack() as ctx:
            ep = ctx.enter_context
            consts = ep(tc.tile_pool(name="consts", bufs=1))
            xpool = ep(tc.tile_pool(name="xslab", bufs=10))
            apool = ep(tc.tile_pool(name="acc", bufs=3))
            spool = ep(tc.tile_pool(name="small", bufs=1))
            scr = ep(tc.tile_pool(name="scratch", bufs=1))
            dram = ep(tc.tile_pool(name="dram", bufs=1, space="DRAM"))
            ppT = ep(tc.tile_pool(name="ppT", bufs=1, space="PSUM"))
            psA = ep(tc.tile_pool(name="psA", bufs=1, space="PSUM"))
            pst = ep(tc.tile_pool(name="pst", bufs=2, space="PSUM"))

            # --- constant loads on the gpsimd SWDGE queue: keeps every HWDGE
            # semaphore lane free for the x stream (HWDGE lanes are shared
            # round-robin and the baseline's x slabs serialized behind the
            # const loads for the first ~40us) ---
            jwin_sb = consts.tile([128, 128], f16, tag="jwin")
            nc.gpsimd.dma_start(jwin_sb[:], jwin)
            ident_sb = consts.tile([128, 128], f16, tag="ident")
            nc.gpsimd.dma_start(ident_sb[:], ident)
            w1_sb = consts.tile([CH, NCHUNK, D1], f16, tag="w1")
            nc.gpsimd.dma_start(w1_sb[:], w1.rearrange("(k p) j -> p k j", p=CH))
            w2_sb = consts.tile([CH, 4, D2], f16, tag="w2")
            nc.gpsimd.dma_start(w2_sb[:], w2.rearrange("(k p) j -> p k j", p=CH))
            b1_sb = consts.tile([1, D1], f16, tag="b1")
            nc.gpsimd.dma_start(b1_sb[:], b1)
            b2_sb = consts.tile([1, D2], f16, tag="b2")
            nc.gpsimd.dma_start(b2_sb[:], b2)
            posm_sb = consts.tile([BPC, B], f32, tag="posm")
            nc.gpsimd.dma_start(posm_sb[:], posm)

            ones_sb = consts.tile([1, BPC], f16, tag="ones")
            nc.vector.memset(ones_sb[:], 1.0)
            mbias = consts.tile([BPC, 1], f32, tag="mbias")
            nc.vector.memset(mbias[:], -INV_T)

            # preload Sqrt/Exp/Ln activation tables while streaming runs
            tld = spool.tile([1, 1], f32, tag="tld")
            nc.vector.memset(tld[:], 1.0)
            tld2 = spool.tile([1, 1], f32, tag="tld2")
            nc.scalar.sqrt(tld2[:], tld[:])
            nc.scalar.activation(tld2[:], tld[:], AF.Exp)
            nc.scalar.activation(tld2[:], tld[:], AF.Ln)

            # --- warm up the collective path early (junk payload, result
            # folded into the output with x0 so it is not dead-code) ---
            wb = dram.tile([BPC, 8], f32, tag="wb")
            wg = dram.tile([B, 8], f32, tag="wg")
            nc.gpsimd.dma_start(wb[:], posm_sb[:, 0:8])
            nc.gpsimd.collective_compute(
                "AllGather", mybir.AluOpType.bypass,
                replica_groups=[list(range(NCORES))],
                ins=[wb.opt()], outs=[wg.opt()],
            )
            wg_sb = spool.tile([BPC, 1], f32, tag="wg")
            nc.gpsimd.dma_start(wg_sb[:], wg[0:BPC, 0:1])

            zb = dram.tile([D2, BPC], f16, tag="zb")
            zg = dram.tile([D2 * NCORES, BPC], f16, tag="zg")

            # --- phase A: stream x fp16, reduce to pooled^T in PSUM ---
            # slab: partition p holds 8 consecutive seq rows of batch row
            # p//64; DVE tree-adds reduce the 8 rows, then 8 chunk matmuls
            # against the sliding one-hot jwin (values 1/512) accumulate
            # pooled^T[d, b] across slabs. All 8 chunk regions share one
            # PSUM bank: start=True clears has_written for the whole bank,
            # so only the very first matmul starts; later chunks' first
            # writes overwrite-where-clear, everything after accumulates.
            pT_ps = ppT.tile([CH, NCHUNK, BPC], f32, tag="pT")
            warm_ps = ppT.tile([BPC, 512], f32, tag="warm")
            for i in range(NSLAB):
                slab = xpool.tile([128, 8, D], f16, tag="slab")
                src = xs[SLAB_B * i:SLAB_B * (i + 1)].rearrange(
                    "b (q m) d -> (b q) m d", m=8)
                nc.sync.dma_start(slab[:], src)
                nc.vector.tensor_tensor(slab[:, 0:4, :], slab[:, 0:4, :],
                                        slab[:, 4:8, :], add)
                nc.vector.tensor_tensor(slab[:, 0:2, :], slab[:, 0:2, :],
                                        slab[:, 2:4, :], add)
                acc = apool.tile([128, D], f16, tag="acc")
                nc.vector.tensor_tensor(acc[:], slab[:, 0, :],
                                        slab[:, 1, :], add)
                jw = jwin_sb[:, 64 - SLAB_B * i:128 - SLAB_B * i]
                for c in range(NCHUNK):
                    nc.tensor.matmul(pT_ps[:, c, :], acc[:, CH * c:CH * (c + 1)],
                                     jw, start=(i == 0 and c == 0),
                                     stop=(i == NSLAB - 1),
                                     skip_group_check=True)
                if i >= NSLAB - 6:
                    # junk matmul reading acc: keeps the PE busy enough at
                    # stream end that HAM unthrottles (2.4 GHz) for the tail
                    nc.tensor.matmul(warm_ps[:], acc[:, 0:BPC], acc[:, 0:512],
                                     start=True, stop=True)

            pT_sb = spool.tile([CH, NCHUNK, BPC], f16, tag="pT")
            nc.vector.tensor_copy(pT_sb[:], pT_ps[:])

            # --- MLP layer 1: h [64, 480] = relu(pooled @ W1 + b1) ---
            h_ps = psA.tile([BPC, D1], f32, tag="h")
            for c in range(NCHUNK):
                nc.tensor.matmul(h_ps[:], pT_sb[:, c, :], w1_sb[:, c, :],
                                 start=(c == 0), stop=False)
            nc.tensor.matmul(h_ps[:], ones_sb[:], b1_sb[:],
                             start=False, stop=True)
            h_sb = spool.tile([BPC, D1], f16, tag="h")
            nc.vector.tensor_scalar_max(h_sb[:], h_ps[:], 0.0)

            # --- h^T via PE transposes ---
            hT_sb = spool.tile([CH, 4, BPC], f16, tag="hT")
            for t in range(4):
                tp = pst.tile([CH, BPC], f16, tag="tp")
                nc.tensor.transpose(tp[:], h_sb[:, CH * t:CH * (t + 1)],
                                    ident_sb[0:BPC, 0:BPC])
                nc.vector.tensor_copy(hT_sb[:, t, :], tp[:])

            # --- MLP layer 2: z [64, 240] = h @ W2 + b2 ---
            z_ps = psA.tile([BPC, D2], f32, tag="z")
            for t in range(4):
                nc.tensor.matmul(z_ps[:], hT_sb[:, t, :], w2_sb[:, t, :],
                                 start=(t == 0), stop=False)
            nc.tensor.matmul(z_ps[:], ones_sb[:], b2_sb[:],
                             start=False, stop=True)

            # --- fold 1/|z| into z: gather unit rows ---
            # (copy z to SBUF first: avoids a double-PSUM-read square and
            # the Square activation table; only Sqrt/Exp/Ln stay resident)
            z_sb = spool.tile([BPC, D2], f32, tag="zsb")
            nc.vector.tensor_copy(z_sb[:], z_ps[:])
            zsq = scr.tile([BPC, D2], f32, tag="zsq")
            nsq = spool.tile([BPC, 1], f32, tag="nsq")
            nc.vector.scalar_tensor_tensor(zsq[:], z_sb[:], 1.0, z_sb[:],
                                           mult, mult, accum_out=nsq[:])
            nrm = spool.tile([BPC, 1], f32, tag="nrm")
            nc.scalar.sqrt(nrm[:], nsq[:])
            ninv = spool.tile([BPC, 1], f32, tag="ninv")
            nc.vector.reciprocal(ninv[:], nrm[:])
            z1_sb = spool.tile([BPC, D2], f16, tag="z1")
            nc.vector.tensor_scalar_mul(z1_sb[:], z_sb[:], ninv[:])

            # --- z1^T [120, 2, 64]; allgather payload rows 0:240 = z1^T ---
            z1T_sb = spool.tile([CH, 2, BPC], f16, tag="z1T")
            for g in range(2):
                tp = pst.tile([CH, BPC], f16, tag="tp")
                nc.tensor.transpose(tp[:], z1_sb[:, CH * g:CH * (g + 1)],
                                    ident_sb[0:BPC, 0:BPC])
                nc.vector.tensor_copy(z1T_sb[:, g, :], tp[:])
            nc.sync.dma_start(
                zb[0:D2, :].rearrange("(og p) b -> p og b", p=CH), z1T_sb[:])
            nc.gpsimd.collective_compute(
                "AllGather", mybir.AluOpType.bypass,
                replica_groups=[list(range(NCORES))],
                ins=[zb.opt()], outs=[zg.opt()],
            )

            # own rows scaled by 1/T while the allgather runs; also the
            # exact diagonal term d2 = (10 z1) . z1 per own row
            z10T_sb = spool.tile([CH, 2, BPC], f16, tag="z10T")
            nc.vector.tensor_scalar_mul(z10T_sb[:], z1T_sb[:], INV_T)
            z10r = spool.tile([BPC, D2], f16, tag="z10r")
            nc.vector.tensor_scalar_mul(z10r[:], z1_sb[:], INV_T)
            dsq = scr.tile([BPC, D2], f32, tag="dsq")
            d2 = spool.tile([BPC, 1], f32, tag="d2")
            nc.vector.scalar_tensor_tensor(dsq[:], z10r[:], 1.0, z1_sb[:],
                                           mult, mult, accum_out=d2[:])
            dexp = spool.tile([BPC, 1], f32, tag="dexp")
            nc.scalar.activation(dexp[:], d2[:], AF.Exp, bias=mbias[:])
            nc.scalar.activation(tld2[:], tld[:], AF.Ln)

            # --- gathered z1^T -> [120, 2, 512] ---
            zgv = zg[:].rearrange("(c r) b -> r c b", r=D2)
            zfT_sb = spool.tile([CH, 2, B], f16, tag="zfT")
            for g in range(2):
                nc.sync.dma_start(
                    zfT_sb[:, g, :].rearrange("p (c b) -> p c b", b=BPC),
                    zgv[CH * g:CH * (g + 1)])

            # --- logits [64, 512] = (10 z1_own) @ z1_all^T, diag == 10 ---
            s_ps = psA.tile([BPC, B], f32, tag="sp")
            nc.tensor.matmul(s_ps[:], z10T_sb[:, 0, :], zfT_sb[:, 0, :],
                             start=True, stop=False)
            nc.tensor.matmul(s_ps[:], z10T_sb[:, 1, :], zfT_sb[:, 1, :],
                             start=False, stop=True)

            # --- nll = -pos + 10 + ln(sum_j exp(l_ij - 10) - diag) ---
            pos = spool.tile([BPC, 1], f32, tag="pos")
            e_sb = scr.tile([BPC, B], f32, tag="esb")
            nc.vector.scalar_tensor_tensor(e_sb[:], s_ps[:], 1.0, posm_sb[:],
                                           mult, mult, accum_out=pos[:])
            esum = spool.tile([BPC, 1], f32, tag="esum")
            e2_sb = scr.tile([BPC, B], f32, tag="e2sb")
            nc.scalar.activation(e2_sb[:], s_ps[:], AF.Exp,
                                 bias=mbias[:], scale=1.0, accum_out=esum[:])
            nc.vector.tensor_tensor(esum[:], esum[:], dexp[:], sub)
            lnv = spool.tile([BPC, 1], f32, tag="lnv")
            nc.scalar.activation(lnv[:], esum[:], AF.Ln)
            nll_sb = spool.tile([BPC, 1], f32, tag="nll")
            nc.vector.tensor_tensor(nll_sb[:], lnv[:], pos[:], sub)
            nc.vector.tensor_scalar_add(nll_sb[:], nll_sb[:], INV_T)
            # keep the warmup collective + PE-warm matmuls alive:
            # nll += 0 * wg + 0 * warm
            nc.vector.scalar_tensor_tensor(nll_sb[:], wg_sb[:], 0.0, nll_sb[:],
                                           mult, add)
            nc.vector.scalar_tensor_tensor(nll_sb[:], warm_ps[:, 0:1], 0.0,
                                           nll_sb[:], mult, add)

            nc.sync.dma_start(out, nll_sb[:])

    nc.compile()
    return nc


def _host_inputs(x, W1c, b1c, W2c, b2c, W1a, b1a, W2a, b2a):
    x16 = np.asarray(x).astype(np.float16)
    # window matrix: slab i selects columns [64-2i, 128-2i); partition p
    # (batch half p//64) hits output column 2i + p//64. 1/512 applies the
    # mean (exact in fp16).
    jwin = np.zeros((128, 128), dtype=np.float16)
    jwin[np.arange(128), 64 + np.arange(128) // 64] = np.float16(1.0 / S)
    ident = np.eye(128, dtype=np.float16)
    in_maps = []
    for c in range(NCORES):
        rows = np.arange(BPC)
        gl = BPC * c + rows
        posm = np.zeros((BPC, B), dtype=np.float32)
        posm[rows, (gl + B // 2) % B] = 1.0
        if c < NCORES // 2:
            w1, bb1, w2, bb2 = W1c, b1c, W2c, b2c
        else:
            w1, bb1, w2, bb2 = W1a, b1a, W2a, b2a
        in_maps.append({
            "xs": x16[BPC * c:BPC * (c + 1)],
            "w1": np.ascontiguousarray(np.asarray(w1).astype(np.float16)),
            "b1": np.asarray(bb1).astype(np.float16).reshape(1, D1),
            "w2": np.ascontiguousarray(np.asarray(w2).astype(np.float16)),
            "b2": np.asarray(bb2).astype(np.float16).reshape(1, D2),
            "jwin": jwin,
            "ident": ident,
            "posm": posm,
        })
    return in_maps


def kernel(x, W1c, b1c, W2c, b2c, W1a, b1a, W2a, b2a):
    global LAST_RESULT
    trace = bool(os.environ.get("BASS_TRACE"))
    if trace:
        _install_ntff_hook()
    from concourse import bass_utils
    if trace:
        bass_utils.upload_artifacts = lambda tmpdir: "local://skipped"

    if "nc" not in _CACHE:
        _CACHE["nc"] = _build_nc()
    nc = _CACHE["nc"]

    in_maps = _host_inputs(x, W1c, b1c, W2c, b2c, W1a, b1a, W2a, b2a)
    kwargs = {}
    if trace:
        kwargs = {"trace": True, "trace_cores": TRACE_CORES}
    res = bass_utils.run_bass_kernel_spmd(
        nc, in_maps, list(range(NCORES)), **kwargs)
    LAST_RESULT = res
    nll = np.concatenate([res.results[c]["nll"][:, 0] for c in range(NCORES)])
    return np.asarray(nll.mean(dtype=np.float64), dtype=np.float32)
